# revision 1
# baseline (speedup 1.0000x reference)
"""Trainium2 Bass kernel for nn_BAGDnet: batched gather + pose-projection.

For each measurement n: look up pose T = tKF[kf_n] (4x4) and map point
p = tMP[mp_n] (xyz), compute q = T @ [p, 1], then perspective-divide and
apply intrinsics:  out[n] = (q.x/q.z*FX + CX, q.y/q.z*FY + CY).

Sharding: data-parallel over the measurement axis across 8 NeuronCores;
tKF/tMP tables are replicated (small).

Per core, gathers use the SWDGE dma_gather instruction (vectorized
descriptor generation, one descriptor per index):
  - pose rows 0..2 gathered directly by keyframe id (48B per index) from a
    256B-stride padded table,
  - map points gathered as 4-point blocks by id>>2 (64B per index; int16
    index range requires the block trick), with the in-block point chosen
    on-chip via predicated copies,
then the projection arithmetic runs on DVE and results stream back.

dma_gather reads its int16 indices wrapped across 16 partitions,
replicated on the 32-partition band belonging to its queue's Q7 pair, so
measurements are processed in 4-tile "supers": the id conversion runs
once on all 128 partitions (4 bands = 4 tiles), and tile b's gathers use
queue b.
"""

import sys

sys.path.insert(0, "/opt/trn_rl_repo")

import math
from contextlib import ExitStack

import numpy as np

from concourse import ap_utils, bacc, bass, mybir
import concourse.tile as tile
from concourse._compat import exact_div
from concourse.bass_utils import run_bass_kernel_spmd

# Tile assigns SWDGE completion-sem lanes (DMASW0..7) round-robin, but a DMA
# semaphore may only ever be updated from one SWDGE queue; with multi-queue
# dma_gather the scheduler's reordering breaks the pairing.  Key the lane on
# the instruction's queue instead.
from concourse import tile_sem_assignment as _tsa


def _install_lane_patch():
    cls = _tsa.TileClockTick
    if getattr(cls, "_queue_lane_patched", False):
        return
    orig = cls._assign_tick

    def patched(self, inst):
        if (
            isinstance(inst, _tsa.DMAInst)
            and inst.engine == mybir.EngineType.Pool
            and not isinstance(inst, _tsa.bass_isa.UserSyncedRemoteDMADescs)
        ):
            q = getattr(inst, "queue_num", None)
            if q is not None:
                self.next_sw_dma_idx = q % self.swdge_sem_count
        return orig(self, inst)

    cls._assign_tick = patched
    cls._queue_lane_patched = True


_install_lane_patch()

FX, FY, CX, CY = 320.0, 320.0, 320.0, 240.0
N_MEAS, N_MP, N_KF = 2_000_000, 100_000, 2_000
N_CORES = 8
PER_CORE = N_MEAS // N_CORES  # 250_000
P = 128
F = 2048  # measurement slots per partition (padded: 128*2048 = 262144/core)
# T*128 indices are buffered as int32 in the 64KB Q7 scratch by the
# dma_gather ucode, so T*128*4B must stay well under 65472B -> T <= ~120.
T = 64  # slots per partition per tile
SUP = 4  # tiles per super-tile (= SWDGE queues)
N_BLK = (N_MP + 3) // 4  # 4-point blocks in the padded point table

f32 = mybir.dt.float32
i32 = mybir.dt.int32
i16 = mybir.dt.int16
Alu = None  # set after import

LAST_RESULTS = None


def _dma_gather_raw(
    gp, out_ap, in_ap, idxs_ap, num_idxs, elem_size, elem_step, queue_num=0,
    single_packet=False,
):
    """bass.BassGpSimd.dma_gather without the elem_size%256B assert.

    The 256B restriction only applies to the transpose path; the
    non-transpose ucode (dma_gather.cpp) requires only that the table row
    STRIDE is a multiple of 256B.  Everything else mirrors bass's method.
    """
    dt = mybir.dt
    assert idxs_ap.dtype == dt.int16
    assert in_ap.dtype == out_ap.dtype
    elem_size_bytes = elem_size * dt.size(in_ap.dtype)
    assert elem_size_bytes > 0 and elem_size_bytes % 4 == 0
    assert in_ap.space == bass.MemorySpace.DRAM
    assert idxs_ap.space == bass.MemorySpace.SBUF
    assert out_ap.space == bass.MemorySpace.SBUF
    assert ap_utils.ap_is_contiguous(out_ap.ap[1:])
    assert ap_utils.ap_is_contiguous(idxs_ap.ap[1:])
    assert in_ap.ap[-1][1] == out_ap.ap[-1][1] == elem_size
    assert out_ap.ap[0][1] * out_ap.ap[1][1] == num_idxs
    assert num_idxs % 128 == 0
    assert in_ap.ap[0][0] == elem_step
    stride_bytes = elem_step * dt.size(in_ap.dtype)
    stride_bytes_256 = exact_div(stride_bytes, 256)
    assert 0 < stride_bytes_256 < 256
    _in_ap = gp.lower_ap_dma(in_ap, for_custom_bir_dma=True)
    _idxs_ap = gp.lower_ap(idxs_ap)
    _out_ap = gp.lower_ap(out_ap)
    return gp.add_instruction(
        mybir.InstDMAGatherAnt(
            name=gp.bass.get_next_instruction_name(),
            ins=[
                *_in_ap,
                _idxs_ap,
                gp.lower_val_access(gp.to_reg(num_idxs)),
            ],
            outs=[_out_ap],
            transpose=False,
            num_idxs=num_idxs,
            elem_size=elem_size,
            stride_bytes_256=stride_bytes_256,
            gen_mode=0,
            # single_packet concatenates each engine's whole descriptor
            # stream into one CME packet; the HW packet ceiling is 64
            # descriptors, so big gathers must use per-descriptor packets.
            single_packet=single_packet,
            queue_num=queue_num,
            sbuf_tokens_per_rank=0,
            sbuf_free_dim_per_rank=0,
            sbuf_free_dim_pad_per_rank=0,
            sbuf_byte_offset=0,
        )
    )


def build_program(F_=F, T_=T, n_kf=N_KF, n_blk=N_BLK, repeat=1, no_compute=False, no_gather=False):
    """Per-core program.  F_ = slots per partition (multiple of 2*T_),
    T_ = tile width.  Two tiles per super: pose gathers of tiles (0,1) use
    queues/bands (0,1), point gathers use queues/bands (2,3), so adjacent
    gathers never share a descriptor ring."""
    A = mybir.AluOpType
    W = T_ * 8  # wrapped width per tile in the [16,*] id layout
    n_tiles = F_ // T_
    assert n_tiles % 2 == 0
    n_sup = n_tiles // 2
    J = F_ * 8  # total wrapped width

    nc = bacc.Bacc(
        "TRN2",
        target_bir_lowering=False,
        debug=False,
        enable_asserts=False,
        num_swdge_queues=SUP,
        # default 16KB rings can't hold a T*128-index gather's descriptors
        # per engine across 4 queues; give SWDGE a 64KB/partition carveout
        dynamic_dma_scratch_size=65536,
    )
    measA = nc.dram_tensor("measA", [16, J, 2], f32, kind="ExternalInput").ap()
    measB = nc.dram_tensor("measB", [P, F_, 2], f32, kind="ExternalInput").ap()
    tkfp = nc.dram_tensor("tkfp", [n_kf, 64], f32, kind="ExternalInput").ap()
    tmpp = nc.dram_tensor("tmpp", [n_blk, 64], f32, kind="ExternalInput").ap()
    out = nc.dram_tensor("out", [P, F_, 2], f32, kind="ExternalOutput").ap()

    with tile.TileContext(nc) as tc, ExitStack() as ctx:
        idxp = ctx.enter_context(tc.tile_pool(name="idx", bufs=2))
        gpool = ctx.enter_context(tc.tile_pool(name="gath", bufs=2))
        iop = ctx.enter_context(tc.tile_pool(name="io", bufs=2))
        wp = ctx.enter_context(tc.tile_pool(name="work", bufs=2))

        for s in [s for _ in range(repeat) for s in range(n_sup)]:
            # --- id conversion for 2 tiles at once: bands 0,1 hold the two
            # tiles' ids for the pose gathers (queues 0,1), bands 2,3 hold
            # the same ids again for the point gathers (queues 2,3); each
            # band is its tile's wrapped ids replicated twice ---
            mA = idxp.tile([P, W, 2], f32, tag="mA")
            for b in range(2):
                t = s * 2 + b
                sl = slice(t * W, (t + 1) * W)
                for r in (0, 16, 64, 80):
                    nc.sync.dma_start(
                        out=mA[32 * b + r : 32 * b + r + 16, :, :],
                        in_=measA[:, sl, :],
                    )
            ki16 = idxp.tile([P, W], i16, tag="ki16")
            nc.vector.tensor_copy(out=ki16[:, :], in_=mA[:, :, 0])
            mi32 = idxp.tile([P, W], i32, tag="mi32")
            nc.vector.tensor_copy(out=mi32[:, :], in_=mA[:, :, 1])
            ms32 = idxp.tile([P, W], i32, tag="ms32")
            nc.vector.tensor_scalar(
                out=ms32[:, :], in0=mi32[:, :],
                scalar1=2, scalar2=None, op0=A.arith_shift_right,
            )
            mb16 = idxp.tile([P, W], i16, tag="mb16")
            nc.vector.tensor_copy(out=mb16[:, :], in_=ms32[:, :])

            for b in range(2):
                t = s * 2 + b
                # --- gathers (one descriptor per measurement each) ---
                pg = gpool.tile([P, T_, 12], f32, tag="pose")
                pb = gpool.tile([P, T_, 16], f32, tag="pblk")
                if not no_gather:
                    _dma_gather_raw(
                        nc.gpsimd, pg[:, :, :], tkfp[:, 0:12], ki16[:, :],
                        num_idxs=T_ * P, elem_size=12, elem_step=64,
                        queue_num=b,
                    )
                    _dma_gather_raw(
                        nc.gpsimd, pb[:, :, :], tmpp[:, 0:16], mb16[:, :],
                        num_idxs=T_ * P, elem_size=16, elem_step=64,
                        queue_num=2 + b,
                    )
                if no_compute:
                    continue

                # --- in-block point select (m%4 via predicated copies) ---
                mB = iop.tile([P, T_, 2], f32, tag="mB")
                nc.sync.dma_start(
                    out=mB[:, :, :], in_=measB[:, t * T_ : (t + 1) * T_, :]
                )
                mi32b = wp.tile([P, T_], i32, tag="mi32b")
                nc.vector.tensor_copy(out=mi32b[:, :], in_=mB[:, :, 1])
                b0 = wp.tile([P, T_], i32, tag="b0")
                nc.vector.tensor_scalar(
                    out=b0[:, :], in0=mi32b[:, :],
                    scalar1=1, scalar2=None, op0=A.bitwise_and,
                )
                b1 = wp.tile([P, T_], i32, tag="b1")
                nc.vector.tensor_scalar(
                    out=b1[:, :], in0=mi32b[:, :],
                    scalar1=1, scalar2=1, op0=A.arith_shift_right,
                    op1=A.bitwise_and,
                )
                # [P,T,4] with a [:, :, 0:3] view keeps the APs 3-D
                # (contiguous [P,T,3] would collapse and break broadcasting)
                pt = wp.tile([P, T_, 4], f32, tag="pt")
                nc.vector.tensor_copy(out=pt[:, :, 0:3], in_=pb[:, :, 0:3])
                nc.vector.copy_predicated(
                    out=pt[:, :, 0:3],
                    mask=b0[:, :].to_broadcast([P, T_, 3]),
                    data=pb[:, :, 4:7],
                )
                hi = wp.tile([P, T_, 4], f32, tag="hi")
                nc.vector.tensor_copy(out=hi[:, :, 0:3], in_=pb[:, :, 8:11])
                nc.vector.copy_predicated(
                    out=hi[:, :, 0:3],
                    mask=b0[:, :].to_broadcast([P, T_, 3]),
                    data=pb[:, :, 12:15],
                )
                nc.vector.copy_predicated(
                    out=pt[:, :, 0:3],
                    mask=b1[:, :].to_broadcast([P, T_, 3]),
                    data=hi[:, :, 0:3],
                )

                # --- projection: rows = T[0:3,:] @ [x,y,z,1] ---
                Xc = pt[:, :, 0]
                Yc = pt[:, :, 1]
                Zc = pt[:, :, 2]
                rows = []
                for r in range(3):
                    acc = wp.tile([P, T_], f32, tag=f"acc{r}")
                    tmp0 = wp.tile([P, T_], f32, tag=f"tmp{r}")
                    nc.vector.tensor_tensor(
                        out=acc[:, :], in0=pg[:, :, 4 * r + 0], in1=Xc,
                        op=A.mult,
                    )
                    nc.vector.tensor_tensor(
                        out=tmp0[:, :], in0=pg[:, :, 4 * r + 1], in1=Yc,
                        op=A.mult,
                    )
                    nc.vector.tensor_tensor(
                        out=acc[:, :], in0=acc[:, :], in1=tmp0[:, :], op=A.add
                    )
                    nc.vector.tensor_tensor(
                        out=tmp0[:, :], in0=pg[:, :, 4 * r + 2], in1=Zc,
                        op=A.mult,
                    )
                    nc.vector.tensor_tensor(
                        out=acc[:, :], in0=acc[:, :], in1=tmp0[:, :], op=A.add
                    )
                    nc.vector.tensor_tensor(
                        out=acc[:, :], in0=acc[:, :], in1=pg[:, :, 4 * r + 3],
                        op=A.add,
                    )
                    rows.append(acc)

                xr, yr, zr = rows
                rz = wp.tile([P, T_], f32, tag="rz")
                nc.vector.reciprocal(out=rz[:, :], in_=zr[:, :])
                xu = wp.tile([P, T_], f32, tag="xu")
                nc.vector.tensor_tensor(
                    out=xu[:, :], in0=xr[:, :], in1=rz[:, :], op=A.mult
                )
                yu = wp.tile([P, T_], f32, tag="yu")
                nc.vector.tensor_tensor(
                    out=yu[:, :], in0=yr[:, :], in1=rz[:, :], op=A.mult
                )

                ot = iop.tile([P, T_, 2], f32, tag="ot")
                nc.vector.tensor_scalar(
                    out=ot[:, :, 0], in0=xu[:, :],
                    scalar1=FX, scalar2=CX, op0=A.mult, op1=A.add,
                )
                nc.vector.tensor_scalar(
                    out=ot[:, :, 1], in0=yu[:, :],
                    scalar1=FY, scalar2=CY, op0=A.mult, op1=A.add,
                )
                nc.sync.dma_start(
                    out=out[:, t * T_ : (t + 1) * T_, :], in_=ot[:, :, :]
                )

    nc.compile()
    return nc


_PROGRAM_CACHE = {}


def _get_program(key, builder):
    if key not in _PROGRAM_CACHE:
        _PROGRAM_CACHE[key] = builder()
    return _PROGRAM_CACHE[key]


def _pack_tables(tMP, tKF):
    n_kf = tKF.shape[0]
    tkfp = np.zeros((n_kf, 64), dtype=np.float32)
    tkfp[:, :16] = tKF.reshape(n_kf, 16)
    n_mp = tMP.shape[0]
    n_blk = (n_mp + 3) // 4
    q = np.zeros((n_blk * 4, 4), dtype=np.float32)
    q[:n_mp, :3] = tMP
    tmpp = np.zeros((n_blk, 64), dtype=np.float32)
    tmpp[:, :16] = q.reshape(n_blk, 16)
    return tkfp, tmpp


def prepare(measurements, tMP, tKF, idxMP, idxKF):
    """Host-side prep: id->index join fallback, sharding, layout.
    Returns (nc, in_maps, per_core)."""
    measurements = np.asarray(measurements, dtype=np.float32)
    tMP = np.ascontiguousarray(np.asarray(tMP, dtype=np.float32))
    tKF = np.ascontiguousarray(np.asarray(tKF, dtype=np.float32))
    idxMP = np.asarray(idxMP)
    idxKF = np.asarray(idxKF)

    n = measurements.shape[0]
    assert n == N_MEAS, f"kernel compiled for {N_MEAS} measurements, got {n}"

    # ids are sorted unique (arange in practice) so the searchsorted join is
    # the identity; otherwise remap on host as a fallback.
    if not (
        idxKF.shape[0] == tKF.shape[0]
        and idxMP.shape[0] == tMP.shape[0]
        and np.array_equal(idxKF, np.arange(idxKF.shape[0], dtype=idxKF.dtype))
        and np.array_equal(idxMP, np.arange(idxMP.shape[0], dtype=idxMP.dtype))
    ):
        ik = np.searchsorted(idxKF, measurements[:, 0].astype(np.int64))
        im = np.searchsorted(idxMP, measurements[:, 1].astype(np.int64))
        measurements = np.stack(
            [ik.astype(np.float32), im.astype(np.float32)], axis=1
        )

    nc = _get_program("main", build_program)
    tkfp, tmpp = _pack_tables(tMP, tKF)

    per = n // N_CORES
    pad_n = P * F
    in_maps = []
    for c in range(N_CORES):
        sl = measurements[c * per : (c + 1) * per]
        if pad_n > per:
            sl = np.concatenate(
                [sl, np.zeros((pad_n - per, 2), dtype=np.float32)], axis=0
            )
        mA = np.ascontiguousarray(sl.reshape(pad_n // 16, 16, 2).transpose(1, 0, 2))
        mB = np.ascontiguousarray(sl.reshape(F, P, 2).transpose(1, 0, 2))
        in_maps.append(
            {"measA": mA, "measB": mB, "tkfp": tkfp, "tmpp": tmpp}
        )
    return nc, in_maps, per


def _assemble(outs_per_core, per):
    res = []
    for o in outs_per_core:
        res.append(o.transpose(1, 0, 2).reshape(P * F, 2)[:per])
    return np.ascontiguousarray(np.concatenate(res, axis=0), dtype=np.float32)


def kernel(measurements, tMP, tKF, idxMP, idxKF, trace=False):
    global LAST_RESULTS
    nc, in_maps, per = prepare(measurements, tMP, tKF, idxMP, idxKF)
    res = run_bass_kernel_spmd(nc, in_maps, list(range(N_CORES)), trace=trace)
    LAST_RESULTS = res
    return _assemble([res.results[c]["out"] for c in range(N_CORES)], per)


# ---------------------------------------------------------------------------
# Timing helpers (devloop only; not used by the grading path)
# ---------------------------------------------------------------------------


def _make_runner(nc, n_cores):
    """Jitted no-donation runner so device-resident inputs can be reused
    across calls.  Modeled on bass2jax.run_bass_via_pjrt."""
    import jax
    from jax.sharding import Mesh, PartitionSpec
    from jax.experimental.shard_map import shard_map
    from concourse.bass2jax import (
        _bass_exec_p,
        install_neuronx_cc_hook,
        partition_id_tensor,
    )

    install_neuronx_cc_hook()
    assert nc.dbg_addr is None
    partition_name = (
        nc.partition_id_tensor.name if nc.partition_id_tensor else None
    )

    in_names, out_names, out_avals = [], [], []
    for alloc in nc.m.functions[0].allocations:
        if not isinstance(alloc, mybir.MemoryLocationSet):
            continue
        name = alloc.memorylocations[0].name
        if alloc.kind == "ExternalInput":
            if name != partition_name:
                in_names.append(name)
        elif alloc.kind == "ExternalOutput":
            out_names.append(name)
            out_avals.append(
                jax.core.ShapedArray(
                    tuple(alloc.tensor_shape), mybir.dt.np(alloc.dtype)
                )
            )
    n_params = len(in_names)
    n_outs = len(out_avals)
    all_names = tuple(
        in_names + out_names + ([partition_name] if partition_name else [])
    )

    def _body(*args):
        extra = [partition_id_tensor()] if partition_name else []
        outs = _bass_exec_p.bind(
            *args,
            *extra,
            out_avals=tuple(out_avals),
            in_names=all_names,
            out_names=tuple(out_names),
            lowering_input_output_aliases=(),
            sim_require_finite=True,
            sim_require_nnan=True,
            nc=nc,
        )
        return tuple(outs)

    devices = jax.devices()[:n_cores]
    mesh = Mesh(np.asarray(devices), ("core",))
    specs = (PartitionSpec("core"),) * (n_params + n_outs)
    fn = jax.jit(
        shard_map(
            _body,
            mesh=mesh,
            in_specs=specs,
            out_specs=(PartitionSpec("core"),) * n_outs,
            check_rep=False,
        ),
        keep_unused=True,
    )
    return fn, mesh, in_names, out_names, out_avals


def run_once_timed(nc, in_maps, reps=5):
    import time
    import jax
    from jax.sharding import NamedSharding, PartitionSpec

    fn, mesh, in_names, out_names, out_avals = _make_runner(nc, len(in_maps))
    n_cores = len(in_maps)
    sh = NamedSharding(mesh, PartitionSpec("core"))
    dev_in = [
        jax.device_put(
            np.concatenate([np.asarray(m[name]) for m in in_maps], axis=0), sh
        )
        for name in in_names
    ]
    dev_zero = [
        jax.device_put(
            np.zeros((n_cores * a.shape[0], *a.shape[1:]), a.dtype), sh
        )
        for a in out_avals
    ]
    out = fn(*dev_in, *dev_zero)  # compile + warm
    jax.block_until_ready(out)
    best = float("inf")
    for _ in range(reps):
        t0 = time.perf_counter()
        out = fn(*dev_in, *dev_zero)
        jax.block_until_ready(out)
        t1 = time.perf_counter()
        best = min(best, t1 - t0)
    return best, [np.asarray(o) for o in out]



# revision 6
# speedup vs baseline: 1.3116x; 1.3116x over previous
"""Trainium2 Bass kernel for nn_BAGDnet: batched gather + pose-projection.

For each measurement n: look up pose T = tKF[kf_n] (4x4) and map point
p = tMP[mp_n] (xyz), compute q = T @ [p, 1], then perspective-divide and
apply intrinsics:  out[n] = (q.x/q.z*FX + CX, q.y/q.z*FY + CY).

Sharding: data-parallel over the measurement axis across 8 NeuronCores;
tKF/tMP tables are replicated (small).

Per core, gathers use the SWDGE dma_gather instruction (vectorized
descriptor generation, one descriptor per index):
  - pose rows 0..2 gathered directly by keyframe id (48B per index) from a
    256B-stride padded table,
  - map points gathered as 4-point blocks by id>>2 (64B per index; int16
    index range requires the block trick), with the in-block point chosen
    on-chip via predicated copies,
then the projection arithmetic runs on DVE and results stream back.

dma_gather reads its int16 indices wrapped across 16 partitions,
replicated on the 32-partition band belonging to its queue's Q7 pair, so
measurements are processed in 4-tile "supers": the id conversion runs
once on all 128 partitions (4 bands = 4 tiles), and tile b's gathers use
queue b.
"""

import sys

sys.path.insert(0, "/opt/trn_rl_repo")

import math
from contextlib import ExitStack

import numpy as np

from concourse import ap_utils, bacc, bass, mybir
import concourse.tile as tile
from concourse._compat import exact_div
from concourse.bass_utils import run_bass_kernel_spmd

# Tile assigns SWDGE completion-sem lanes (DMASW0..7) round-robin, but a DMA
# semaphore may only ever be updated from one SWDGE queue; with multi-queue
# dma_gather the scheduler's reordering breaks the pairing.  Key the lane on
# the instruction's queue instead.
from concourse import tile_sem_assignment as _tsa


def _install_lane_patch():
    cls = _tsa.TileClockTick
    if getattr(cls, "_queue_lane_patched", False):
        return
    orig = cls._assign_tick

    def patched(self, inst):
        if (
            isinstance(inst, _tsa.DMAInst)
            and inst.engine == mybir.EngineType.Pool
            and not isinstance(inst, _tsa.bass_isa.UserSyncedRemoteDMADescs)
        ):
            q = getattr(inst, "queue_num", None)
            if q is not None:
                self.next_sw_dma_idx = q % self.swdge_sem_count
        return orig(self, inst)

    cls._assign_tick = patched
    cls._queue_lane_patched = True


_install_lane_patch()

FX, FY, CX, CY = 320.0, 320.0, 320.0, 240.0
N_MEAS, N_MP, N_KF = 2_000_000, 100_000, 2_000
N_CORES = 8
PER_CORE = N_MEAS // N_CORES  # 250_000
P = 128
F = 2048  # measurement slots per partition (padded: 128*2048 = 262144/core)
# T*128 indices are buffered as int32 in the 64KB Q7 scratch by the
# dma_gather ucode, so T*128*4B must stay well under 65472B -> T <= ~120.
T = 64  # slots per partition per tile
SUP = 4  # tiles per super-tile (= SWDGE queues)
N_BLK = (N_MP + 3) // 4  # 4-point blocks in the padded point table

f32 = mybir.dt.float32
i32 = mybir.dt.int32
i16 = mybir.dt.int16
Alu = None  # set after import

LAST_RESULTS = None


def _dma_gather_raw(
    gp, out_ap, in_ap, idxs_ap, num_idxs, elem_size, elem_step, queue_num=0,
    single_packet=False,
):
    """bass.BassGpSimd.dma_gather without the elem_size%256B assert.

    The 256B restriction only applies to the transpose path; the
    non-transpose ucode (dma_gather.cpp) requires only that the table row
    STRIDE is a multiple of 256B.  Everything else mirrors bass's method.
    """
    dt = mybir.dt
    assert idxs_ap.dtype == dt.int16
    assert in_ap.dtype == out_ap.dtype
    elem_size_bytes = elem_size * dt.size(in_ap.dtype)
    assert elem_size_bytes > 0 and elem_size_bytes % 4 == 0
    assert in_ap.space == bass.MemorySpace.DRAM
    assert idxs_ap.space == bass.MemorySpace.SBUF
    assert out_ap.space == bass.MemorySpace.SBUF
    assert ap_utils.ap_is_contiguous(out_ap.ap[1:])
    assert ap_utils.ap_is_contiguous(idxs_ap.ap[1:])
    assert in_ap.ap[-1][1] == out_ap.ap[-1][1] == elem_size
    assert out_ap.ap[0][1] * out_ap.ap[1][1] == num_idxs
    assert num_idxs % 128 == 0
    assert in_ap.ap[0][0] == elem_step
    stride_bytes = elem_step * dt.size(in_ap.dtype)
    stride_bytes_256 = exact_div(stride_bytes, 256)
    assert 0 < stride_bytes_256 < 256
    _in_ap = gp.lower_ap_dma(in_ap, for_custom_bir_dma=True)
    _idxs_ap = gp.lower_ap(idxs_ap)
    _out_ap = gp.lower_ap(out_ap)
    return gp.add_instruction(
        mybir.InstDMAGatherAnt(
            name=gp.bass.get_next_instruction_name(),
            ins=[
                *_in_ap,
                _idxs_ap,
                gp.lower_val_access(gp.to_reg(num_idxs)),
            ],
            outs=[_out_ap],
            transpose=False,
            num_idxs=num_idxs,
            elem_size=elem_size,
            stride_bytes_256=stride_bytes_256,
            gen_mode=0,
            # single_packet concatenates each engine's whole descriptor
            # stream into one CME packet; the HW packet ceiling is 64
            # descriptors, so big gathers must use per-descriptor packets.
            single_packet=single_packet,
            queue_num=queue_num,
            sbuf_tokens_per_rank=0,
            sbuf_free_dim_per_rank=0,
            sbuf_free_dim_pad_per_rank=0,
            sbuf_byte_offset=0,
        )
    )


def build_program(F_=F, T_=T, n_kf=N_KF, n_blk=N_BLK, repeat=1, no_compute=False, no_gather=False, gbufs=2, no_stream=False):
    """Per-core program.  F_ = slots per partition (multiple of 2*T_),
    T_ = tile width.  Two tiles per super: pose gathers of tiles (0,1) use
    queues/bands (0,1), point gathers use queues/bands (2,3), so adjacent
    gathers never share a descriptor ring."""
    A = mybir.AluOpType
    W = T_ * 8  # wrapped width per tile in the [16,*] id layout
    n_tiles = F_ // T_
    assert n_tiles % 2 == 0
    n_sup = n_tiles // 2
    J = F_ * 8  # total wrapped width

    nc = bacc.Bacc(
        "TRN2",
        target_bir_lowering=False,
        debug=False,
        enable_asserts=False,
        num_swdge_queues=SUP,
        # default 16KB rings can't hold a T*128-index gather's descriptors
        # per engine across 4 queues; give SWDGE a 64KB/partition carveout
        dynamic_dma_scratch_size=65536,
    )
    measA = nc.dram_tensor("measA", [16, J, 2], f32, kind="ExternalInput").ap()
    measB = nc.dram_tensor("measB", [P, F_, 2], f32, kind="ExternalInput").ap()
    tkfp = nc.dram_tensor("tkfp", [n_kf, 64], f32, kind="ExternalInput").ap()
    tmpp = nc.dram_tensor("tmpp", [n_blk, 64], f32, kind="ExternalInput").ap()
    out = nc.dram_tensor("out", [P, F_, 2], f32, kind="ExternalOutput").ap()

    with tile.TileContext(nc) as tc, ExitStack() as ctx:
        idxp = ctx.enter_context(tc.tile_pool(name="idx", bufs=2))
        gpool = ctx.enter_context(tc.tile_pool(name="gath", bufs=gbufs))
        iop = ctx.enter_context(tc.tile_pool(name="io", bufs=2))
        wp = ctx.enter_context(tc.tile_pool(name="work", bufs=2))

        if no_stream:
            # pure-gather probe: one static idx tile pair, no per-super
            # streaming; same gather count/queues as the real kernel
            sp = ctx.enter_context(tc.tile_pool(name="static", bufs=1))
            ki16s = sp.tile([P, T_ * 8], i16, tag="ki16s")
            mb16s = sp.tile([P, T_ * 8], i16, tag="mb16s")
            mA0 = sp.tile([P, T_ * 8, 2], f32, tag="mA0")
            for r in (0, 16, 32, 48, 64, 80, 96, 112):
                nc.sync.dma_start(
                    out=mA0[r : r + 16, :, :], in_=measA[:, 0 : T_ * 8, :]
                )
            nc.vector.tensor_copy(out=ki16s[:, :], in_=mA0[:, :, 0])
            mi32s = sp.tile([P, T_ * 8], i32, tag="mi32s")
            nc.vector.tensor_copy(out=mi32s[:, :], in_=mA0[:, :, 1])
            nc.vector.tensor_scalar(
                out=mi32s[:, :], in0=mi32s[:, :],
                scalar1=2, scalar2=None, op0=mybir.AluOpType.arith_shift_right,
            )
            nc.vector.tensor_copy(out=mb16s[:, :], in_=mi32s[:, :])
            for s in [s for _ in range(repeat) for s in range(n_sup)]:
                for b in range(2):
                    pg = gpool.tile([P, T_, 12], f32, tag="pose")
                    pb = gpool.tile([P, T_, 16], f32, tag="pblk")
                    _dma_gather_raw(
                        nc.gpsimd, pg[:, :, :], tkfp[:, 0:12], ki16s[:, 0 : T_ * 8],
                        num_idxs=T_ * P, elem_size=12, elem_step=64,
                        queue_num=b,
                    )
                    _dma_gather_raw(
                        nc.gpsimd, pb[:, :, :], tmpp[:, 0:16], mb16s[:, 0 : T_ * 8],
                        num_idxs=T_ * P, elem_size=16, elem_step=64,
                        queue_num=2 + b,
                    )
            ot = iop.tile([P, T_, 2], f32, tag="ot")
            nc.vector.tensor_copy(out=ot[:, :, 0], in_=pg[:, :, 0])
            nc.vector.tensor_copy(out=ot[:, :, 1], in_=pb[:, :, 0])
            nc.sync.dma_start(out=out[:, 0:T_, :], in_=ot[:, :, :])
            nc.compile()
            return nc

        for s in [s for _ in range(repeat) for s in range(n_sup)]:
            # --- id conversion for 2 tiles at once: bands 0,1 hold the two
            # tiles' ids for the pose gathers (queues 0,1), bands 2,3 hold
            # the same ids again for the point gathers (queues 2,3); each
            # band is its tile's wrapped ids replicated twice ---
            mA = idxp.tile([P, W, 2], f32, tag="mA")
            for b in range(2):
                t = s * 2 + b
                sl = slice(t * W, (t + 1) * W)
                for r in (0, 16, 64, 80):
                    nc.sync.dma_start(
                        out=mA[32 * b + r : 32 * b + r + 16, :, :],
                        in_=measA[:, sl, :],
                    )
            ki16 = idxp.tile([P, W], i16, tag="ki16")
            nc.vector.tensor_copy(out=ki16[:, :], in_=mA[:, :, 0])
            mi32 = idxp.tile([P, W], i32, tag="mi32")
            nc.vector.tensor_copy(out=mi32[:, :], in_=mA[:, :, 1])
            ms32 = idxp.tile([P, W], i32, tag="ms32")
            nc.vector.tensor_scalar(
                out=ms32[:, :], in0=mi32[:, :],
                scalar1=2, scalar2=None, op0=A.arith_shift_right,
            )
            mb16 = idxp.tile([P, W], i16, tag="mb16")
            nc.vector.tensor_copy(out=mb16[:, :], in_=ms32[:, :])

            for b in range(2):
                t = s * 2 + b
                # --- gathers (one descriptor per measurement each) ---
                pg = gpool.tile([P, T_, 12], f32, tag="pose")
                pb = gpool.tile([P, T_, 16], f32, tag="pblk")
                if not no_gather:
                    _dma_gather_raw(
                        nc.gpsimd, pg[:, :, :], tkfp[:, 0:12], ki16[:, :],
                        num_idxs=T_ * P, elem_size=12, elem_step=64,
                        queue_num=b,
                    )
                    _dma_gather_raw(
                        nc.gpsimd, pb[:, :, :], tmpp[:, 0:16], mb16[:, :],
                        num_idxs=T_ * P, elem_size=16, elem_step=64,
                        queue_num=2 + b,
                    )
                if no_compute:
                    continue

                # --- in-block point select (m%4 via predicated copies) ---
                mB = iop.tile([P, T_, 2], f32, tag="mB")
                nc.sync.dma_start(
                    out=mB[:, :, :], in_=measB[:, t * T_ : (t + 1) * T_, :]
                )
                mi32b = wp.tile([P, T_], i32, tag="mi32b")
                nc.vector.tensor_copy(out=mi32b[:, :], in_=mB[:, :, 1])
                b0 = wp.tile([P, T_], i32, tag="b0")
                nc.vector.tensor_scalar(
                    out=b0[:, :], in0=mi32b[:, :],
                    scalar1=1, scalar2=None, op0=A.bitwise_and,
                )
                b1 = wp.tile([P, T_], i32, tag="b1")
                nc.vector.tensor_scalar(
                    out=b1[:, :], in0=mi32b[:, :],
                    scalar1=1, scalar2=1, op0=A.arith_shift_right,
                    op1=A.bitwise_and,
                )
                # [P,T,4] with a [:, :, 0:3] view keeps the APs 3-D
                # (contiguous [P,T,3] would collapse and break broadcasting)
                pt = wp.tile([P, T_, 4], f32, tag="pt")
                nc.vector.tensor_copy(out=pt[:, :, 0:3], in_=pb[:, :, 0:3])
                nc.vector.copy_predicated(
                    out=pt[:, :, 0:3],
                    mask=b0[:, :].to_broadcast([P, T_, 3]),
                    data=pb[:, :, 4:7],
                )
                hi = wp.tile([P, T_, 4], f32, tag="hi")
                nc.vector.tensor_copy(out=hi[:, :, 0:3], in_=pb[:, :, 8:11])
                nc.vector.copy_predicated(
                    out=hi[:, :, 0:3],
                    mask=b0[:, :].to_broadcast([P, T_, 3]),
                    data=pb[:, :, 12:15],
                )
                nc.vector.copy_predicated(
                    out=pt[:, :, 0:3],
                    mask=b1[:, :].to_broadcast([P, T_, 3]),
                    data=hi[:, :, 0:3],
                )

                # --- projection: rows = T[0:3,:] @ [x,y,z,1] ---
                Xc = pt[:, :, 0]
                Yc = pt[:, :, 1]
                Zc = pt[:, :, 2]
                rows = []
                for r in range(3):
                    acc = wp.tile([P, T_], f32, tag=f"acc{r}")
                    tmp0 = wp.tile([P, T_], f32, tag=f"tmp{r}")
                    nc.vector.tensor_tensor(
                        out=acc[:, :], in0=pg[:, :, 4 * r + 0], in1=Xc,
                        op=A.mult,
                    )
                    nc.vector.tensor_tensor(
                        out=tmp0[:, :], in0=pg[:, :, 4 * r + 1], in1=Yc,
                        op=A.mult,
                    )
                    nc.vector.tensor_tensor(
                        out=acc[:, :], in0=acc[:, :], in1=tmp0[:, :], op=A.add
                    )
                    nc.vector.tensor_tensor(
                        out=tmp0[:, :], in0=pg[:, :, 4 * r + 2], in1=Zc,
                        op=A.mult,
                    )
                    nc.vector.tensor_tensor(
                        out=acc[:, :], in0=acc[:, :], in1=tmp0[:, :], op=A.add
                    )
                    nc.vector.tensor_tensor(
                        out=acc[:, :], in0=acc[:, :], in1=pg[:, :, 4 * r + 3],
                        op=A.add,
                    )
                    rows.append(acc)

                xr, yr, zr = rows
                rz = wp.tile([P, T_], f32, tag="rz")
                nc.vector.reciprocal(out=rz[:, :], in_=zr[:, :])
                xu = wp.tile([P, T_], f32, tag="xu")
                nc.vector.tensor_tensor(
                    out=xu[:, :], in0=xr[:, :], in1=rz[:, :], op=A.mult
                )
                yu = wp.tile([P, T_], f32, tag="yu")
                nc.vector.tensor_tensor(
                    out=yu[:, :], in0=yr[:, :], in1=rz[:, :], op=A.mult
                )

                ot = iop.tile([P, T_, 2], f32, tag="ot")
                nc.vector.tensor_scalar(
                    out=ot[:, :, 0], in0=xu[:, :],
                    scalar1=FX, scalar2=CX, op0=A.mult, op1=A.add,
                )
                nc.vector.tensor_scalar(
                    out=ot[:, :, 1], in0=yu[:, :],
                    scalar1=FY, scalar2=CY, op0=A.mult, op1=A.add,
                )
                nc.sync.dma_start(
                    out=out[:, t * T_ : (t + 1) * T_, :], in_=ot[:, :, :]
                )

    nc.compile()
    return nc


_PROGRAM_CACHE = {}


def _get_program(key, builder):
    if key not in _PROGRAM_CACHE:
        _PROGRAM_CACHE[key] = builder()
    return _PROGRAM_CACHE[key]


def _pack_tables(tMP, tKF):
    n_kf = tKF.shape[0]
    tkfp = np.zeros((n_kf, 64), dtype=np.float32)
    tkfp[:, :16] = tKF.reshape(n_kf, 16)
    n_mp = tMP.shape[0]
    n_blk = (n_mp + 3) // 4
    q = np.zeros((n_blk * 4, 4), dtype=np.float32)
    q[:n_mp, :3] = tMP
    tmpp = np.zeros((n_blk, 64), dtype=np.float32)
    tmpp[:, :16] = q.reshape(n_blk, 16)
    return tkfp, tmpp


def prepare(measurements, tMP, tKF, idxMP, idxKF):
    """Host-side prep: id->index join fallback, sharding, layout.
    Returns (nc, in_maps, per_core)."""
    measurements = np.asarray(measurements, dtype=np.float32)
    tMP = np.ascontiguousarray(np.asarray(tMP, dtype=np.float32))
    tKF = np.ascontiguousarray(np.asarray(tKF, dtype=np.float32))
    idxMP = np.asarray(idxMP)
    idxKF = np.asarray(idxKF)

    n = measurements.shape[0]
    assert n == N_MEAS, f"kernel compiled for {N_MEAS} measurements, got {n}"

    # ids are sorted unique (arange in practice) so the searchsorted join is
    # the identity; otherwise remap on host as a fallback.
    if not (
        idxKF.shape[0] == tKF.shape[0]
        and idxMP.shape[0] == tMP.shape[0]
        and np.array_equal(idxKF, np.arange(idxKF.shape[0], dtype=idxKF.dtype))
        and np.array_equal(idxMP, np.arange(idxMP.shape[0], dtype=idxMP.dtype))
    ):
        ik = np.searchsorted(idxKF, measurements[:, 0].astype(np.int64))
        im = np.searchsorted(idxMP, measurements[:, 1].astype(np.int64))
        measurements = np.stack(
            [ik.astype(np.float32), im.astype(np.float32)], axis=1
        )

    nc = _get_program("main", build_program)
    tkfp, tmpp = _pack_tables(tMP, tKF)

    per = n // N_CORES
    pad_n = P * F
    in_maps = []
    for c in range(N_CORES):
        sl = measurements[c * per : (c + 1) * per]
        if pad_n > per:
            sl = np.concatenate(
                [sl, np.zeros((pad_n - per, 2), dtype=np.float32)], axis=0
            )
        mA = np.ascontiguousarray(sl.reshape(pad_n // 16, 16, 2).transpose(1, 0, 2))
        mB = np.ascontiguousarray(sl.reshape(F, P, 2).transpose(1, 0, 2))
        in_maps.append(
            {"measA": mA, "measB": mB, "tkfp": tkfp, "tmpp": tmpp}
        )
    return nc, in_maps, per


def _assemble(outs_per_core, per):
    res = []
    for o in outs_per_core:
        res.append(o.transpose(1, 0, 2).reshape(P * F, 2)[:per])
    return np.ascontiguousarray(np.concatenate(res, axis=0), dtype=np.float32)


def kernel(measurements, tMP, tKF, idxMP, idxKF, trace=False):
    global LAST_RESULTS
    nc, in_maps, per = prepare(measurements, tMP, tKF, idxMP, idxKF)
    res = run_bass_kernel_spmd(nc, in_maps, list(range(N_CORES)), trace=trace)
    LAST_RESULTS = res
    return _assemble([res.results[c]["out"] for c in range(N_CORES)], per)


# ---------------------------------------------------------------------------
# Timing helpers (devloop only; not used by the grading path)
# ---------------------------------------------------------------------------


def _make_runner(nc, n_cores):
    """Jitted no-donation runner so device-resident inputs can be reused
    across calls.  Modeled on bass2jax.run_bass_via_pjrt."""
    import jax
    from jax.sharding import Mesh, PartitionSpec
    from jax.experimental.shard_map import shard_map
    from concourse.bass2jax import (
        _bass_exec_p,
        install_neuronx_cc_hook,
        partition_id_tensor,
    )

    install_neuronx_cc_hook()
    assert nc.dbg_addr is None
    partition_name = (
        nc.partition_id_tensor.name if nc.partition_id_tensor else None
    )

    in_names, out_names, out_avals = [], [], []
    for alloc in nc.m.functions[0].allocations:
        if not isinstance(alloc, mybir.MemoryLocationSet):
            continue
        name = alloc.memorylocations[0].name
        if alloc.kind == "ExternalInput":
            if name != partition_name:
                in_names.append(name)
        elif alloc.kind == "ExternalOutput":
            out_names.append(name)
            out_avals.append(
                jax.core.ShapedArray(
                    tuple(alloc.tensor_shape), mybir.dt.np(alloc.dtype)
                )
            )
    n_params = len(in_names)
    n_outs = len(out_avals)
    all_names = tuple(
        in_names + out_names + ([partition_name] if partition_name else [])
    )

    def _body(*args):
        extra = [partition_id_tensor()] if partition_name else []
        outs = _bass_exec_p.bind(
            *args,
            *extra,
            out_avals=tuple(out_avals),
            in_names=all_names,
            out_names=tuple(out_names),
            lowering_input_output_aliases=(),
            sim_require_finite=True,
            sim_require_nnan=True,
            nc=nc,
        )
        return tuple(outs)

    devices = jax.devices()[:n_cores]
    mesh = Mesh(np.asarray(devices), ("core",))
    specs = (PartitionSpec("core"),) * (n_params + n_outs)
    fn = jax.jit(
        shard_map(
            _body,
            mesh=mesh,
            in_specs=specs,
            out_specs=(PartitionSpec("core"),) * n_outs,
            check_rep=False,
        ),
        keep_unused=True,
    )
    return fn, mesh, in_names, out_names, out_avals


def run_once_timed(nc, in_maps, reps=5):
    import time
    import jax
    from jax.sharding import NamedSharding, PartitionSpec

    fn, mesh, in_names, out_names, out_avals = _make_runner(nc, len(in_maps))
    n_cores = len(in_maps)
    sh = NamedSharding(mesh, PartitionSpec("core"))
    dev_in = [
        jax.device_put(
            np.concatenate([np.asarray(m[name]) for m in in_maps], axis=0), sh
        )
        for name in in_names
    ]
    dev_zero = [
        jax.device_put(
            np.zeros((n_cores * a.shape[0], *a.shape[1:]), a.dtype), sh
        )
        for a in out_avals
    ]
    out = fn(*dev_in, *dev_zero)  # compile + warm
    jax.block_until_ready(out)
    best = float("inf")
    for _ in range(reps):
        t0 = time.perf_counter()
        out = fn(*dev_in, *dev_zero)
        jax.block_until_ready(out)
        t1 = time.perf_counter()
        best = min(best, t1 - t0)
    return best, [np.asarray(o) for o in out]

